# revision 3
# baseline (speedup 1.0000x reference)
"""HAN metapath-attention kernel for 8 Trainium2 NeuronCores.

Problem (per reference):
    inputs        [B=512, P=64, D=512] f32   job embeddings
    title_emb_mat [T=50000, D=512]     f32   title table
    nbr_batch/job/title/mask [B, P, K=8] i32 neighbor indices + mask

    ref_embs = inputs[nbr_batch, nbr_job]           # gather over full batch
    raw_s    = einsum('bpd,bpkd->bpk', inputs, ref_embs)
    sim      = softmax(where(mask, raw_s, -1e9)) * mask
    out      = concat([inputs, einsum('bpk,bpkt->bpt', sim, title[nbr_title])], -1)
    returns [B*P, 2D]

Sharding: data-parallel over flat rows r = b*P + p; core c owns rows
[c*4096, (c+1)*4096). `inputs` (flattened to a [32768, 512] gather table) and
`title_emb_mat` are replicated to every core; neighbor indices are
host-flattened to global row ids so gathers need no cross-core traffic.

Per core, per group of 128 focal rows:
  - focal tile F loaded sequentially (HWDGE)
  - all 8 neighbor job rows per focal gathered in ONE dma_gather
    (1024 int16 indices; job row ids fit int16 since B*P == 32768)
  - 8 title-row gathers via indirect_dma_start (int32 ids, one row/partition)
  - dots via fused scalar_tensor_tensor (product + row-reduce accum_out)
  - masked softmax over k in the free dim (mask folded in as +0/-1e9 bias)
  - weighted title sum via fused scalar_tensor_tensor (mult+add) chain
  - store [128, 0:512]=F and [128, 512:1024]=acc
"""

import os
import sys
import time

if "/opt/trn_rl_repo" not in sys.path:
    sys.path.insert(0, "/opt/trn_rl_repo")

import numpy as np

import concourse.bacc as bacc
import concourse.bass as bass
import concourse.tile as tile
from concourse import mybir

B, P, K, D, T = 512, 64, 8, 512, 50000
NCORES = 8
R = B * P // NCORES  # 4096 focal rows per core
G = R // 128  # 32 groups of 128 focal rows
F32 = mybir.dt.float32
I32 = mybir.dt.int32
I16 = mybir.dt.int16


def _build_program(niter=1):
    nc = bacc.Bacc("TRN2", target_bir_lowering=False, debug=False)

    emb = nc.dram_tensor("emb", [B * P, D], F32, kind="ExternalInput")
    title = nc.dram_tensor("title", [T, D], F32, kind="ExternalInput")
    focal = nc.dram_tensor("focal", [R, D], F32, kind="ExternalInput")
    jidx16 = nc.dram_tensor("jidx16", [128, G * 64], I16, kind="ExternalInput")
    tidx = nc.dram_tensor("tidx", [128, G * K], I32, kind="ExternalInput")
    maskf = nc.dram_tensor("maskf", [128, G * K], F32, kind="ExternalInput")
    maskb = nc.dram_tensor("maskb", [128, G * K], F32, kind="ExternalInput")
    outp = nc.dram_tensor("out", [R, 2 * D], F32, kind="ExternalOutput")

    with tile.TileContext(nc) as tc:
        with (
            tc.tile_pool(name="idxp", bufs=1) as idxp,
            tc.tile_pool(name="fp", bufs=4) as fp,
            tc.tile_pool(name="jp", bufs=3) as jp,
            tc.tile_pool(name="tp", bufs=3) as tp,
            tc.tile_pool(name="wp", bufs=3) as wp,
            tc.tile_pool(name="sp", bufs=4) as sp,
        ):
            jt = idxp.tile([128, G * 64], I16)
            tt = idxp.tile([128, G * K], I32)
            mf = idxp.tile([128, G * K], F32)
            mb = idxp.tile([128, G * K], F32)
            nc.sync.dma_start(out=jt[:], in_=jidx16[:])
            nc.sync.dma_start(out=tt[:], in_=tidx[:])
            nc.sync.dma_start(out=mf[:], in_=maskf[:])
            nc.sync.dma_start(out=mb[:], in_=maskb[:])

            for _ in range(niter):
                for g in range(G):
                    rows = slice(g * 128, (g + 1) * 128)
                    F = fp.tile([128, D], F32, tag="F")
                    nc.sync.dma_start(out=F[:], in_=focal[rows, :])

                    J = jp.tile([128, K, D], F32, tag="J")
                    nc.gpsimd.dma_gather(
                        J[:], emb[:], jt[:, g * 64 : (g + 1) * 64], 1024, 1024, D
                    )

                    Tt = tp.tile([128, K, D], F32, tag="T")
                    for k in range(K):
                        nc.gpsimd.indirect_dma_start(
                            out=Tt[:, k, :],
                            out_offset=None,
                            in_=title[:],
                            in_offset=bass.IndirectOffsetOnAxis(
                                ap=tt[:, g * K + k : g * K + k + 1], axis=0
                            ),
                        )

                    # dots[:, k] = sum_d F * J_k (fused product + row-reduce)
                    dots = sp.tile([128, K], F32, tag="dots")
                    prod = wp.tile([128, D], F32, tag="prod")
                    for k in range(K):
                        nc.vector.scalar_tensor_tensor(
                            out=prod[:],
                            in0=F[:],
                            scalar=1.0,
                            in1=J[:, k, :],
                            op0=mybir.AluOpType.mult,
                            op1=mybir.AluOpType.mult,
                            accum_out=dots[:, k : k + 1],
                        )
                    # masked logits = dots + maskbias (0 or -1e9)
                    logits = sp.tile([128, K], F32, tag="logits")
                    nc.vector.tensor_tensor(
                        out=logits[:],
                        in0=dots[:],
                        in1=mb[:, g * K : (g + 1) * K],
                        op=mybir.AluOpType.add,
                    )

                    negM = sp.tile([128, 1], F32, tag="negM")
                    nc.vector.tensor_reduce(
                        out=negM[:],
                        in_=logits[:],
                        axis=mybir.AxisListType.X,
                        op=mybir.AluOpType.max,
                        negate=True,
                    )
                    e = sp.tile([128, K], F32, tag="e")
                    nc.scalar.activation(
                        out=e[:],
                        in_=logits[:],
                        func=mybir.ActivationFunctionType.Exp,
                        bias=negM[:, 0:1],
                        scale=1.0,
                    )
                    ssum = sp.tile([128, 1], F32, tag="ssum")
                    nc.vector.tensor_reduce(
                        out=ssum[:],
                        in_=e[:],
                        axis=mybir.AxisListType.X,
                        op=mybir.AluOpType.add,
                    )
                    rr = sp.tile([128, 1], F32, tag="rr")
                    nc.vector.reciprocal(out=rr[:], in_=ssum[:])
                    sim = sp.tile([128, K], F32, tag="sim")
                    nc.vector.scalar_tensor_tensor(
                        out=sim[:],
                        in0=e[:],
                        scalar=rr[:, 0:1],
                        in1=mf[:, g * K : (g + 1) * K],
                        op0=mybir.AluOpType.mult,
                        op1=mybir.AluOpType.mult,
                    )

                    acc = wp.tile([128, D], F32, tag="acc")
                    nc.vector.tensor_scalar_mul(
                        out=acc[:], in0=Tt[:, 0, :], scalar1=sim[:, 0:1]
                    )
                    for k in range(1, K):
                        nc.vector.scalar_tensor_tensor(
                            out=acc[:],
                            in0=Tt[:, k, :],
                            scalar=sim[:, k : k + 1],
                            in1=acc[:],
                            op0=mybir.AluOpType.mult,
                            op1=mybir.AluOpType.add,
                        )

                    nc.sync.dma_start(out=outp[rows, 0:D], in_=F[:])
                    nc.sync.dma_start(out=outp[rows, D : 2 * D], in_=acc[:])

    nc.finalize()
    return nc


# ---------------------------------------------------------------- runner ----

_RUNNERS = {}


class _Runner:
    """Caches the sharded jit executable for one program variant so repeated
    executions skip retracing/recompiling (adapted from
    concourse.bass2jax.run_bass_via_pjrt's multi-core branch)."""

    def __init__(self, niter):
        import jax
        from jax.experimental.shard_map import shard_map
        from jax.sharding import Mesh, NamedSharding, PartitionSpec

        from concourse import mybir as _mb
        from concourse.bass2jax import (
            _bass_exec_p,
            install_neuronx_cc_hook,
            partition_id_tensor,
        )

        install_neuronx_cc_hook()
        self.jax = jax
        nc = _build_program(niter)
        self.nc = nc

        in_names, out_names, out_avals = [], [], []
        partition_name = (
            nc.partition_id_tensor.name if nc.partition_id_tensor else None
        )
        for alloc in nc.m.functions[0].allocations:
            if not isinstance(alloc, _mb.MemoryLocationSet):
                continue
            name = alloc.memorylocations[0].name
            if alloc.kind == "ExternalInput":
                if name != partition_name:
                    in_names.append(name)
            elif alloc.kind == "ExternalOutput":
                shape = tuple(alloc.tensor_shape)
                dtype = _mb.dt.np(alloc.dtype)
                out_names.append(name)
                out_avals.append(jax.core.ShapedArray(shape, dtype))

        self.in_names = list(in_names)
        self.out_names = out_names
        self.out_avals = out_avals
        n_params = len(in_names)
        n_outs = len(out_avals)

        bind_in_names = list(in_names) + list(out_names)
        if partition_name is not None:
            bind_in_names.append(partition_name)

        def _body(*args):
            operands = list(args)
            if partition_name is not None:
                operands.append(partition_id_tensor())
            outs = _bass_exec_p.bind(
                *operands,
                out_avals=tuple(out_avals),
                in_names=tuple(bind_in_names),
                out_names=tuple(out_names),
                lowering_input_output_aliases=(),
                sim_require_finite=True,
                sim_require_nnan=True,
                nc=nc,
            )
            return tuple(outs)

        devices = jax.devices()[:NCORES]
        mesh = Mesh(np.asarray(devices), ("core",))
        self.mesh = mesh
        self.sharding = NamedSharding(mesh, PartitionSpec("core"))
        in_specs = (PartitionSpec("core"),) * (n_params + n_outs)
        out_specs = (PartitionSpec("core"),) * n_outs
        donate = tuple(range(n_params, n_params + n_outs))
        self.fn = jax.jit(
            shard_map(
                _body,
                mesh=mesh,
                in_specs=in_specs,
                out_specs=out_specs,
                check_rep=False,
            ),
            donate_argnums=donate,
            keep_unused=True,
        )

    def place_inputs(self, in_maps):
        """device_put the concatenated per-core inputs once."""
        concat = [
            np.concatenate([np.asarray(m[name]) for m in in_maps], axis=0)
            for name in self.in_names
        ]
        return [self.jax.device_put(a, self.sharding) for a in concat]

    def make_zeros(self):
        return [
            self.jax.device_put(
                np.zeros((NCORES * av.shape[0], *av.shape[1:]), av.dtype),
                self.sharding,
            )
            for av in self.out_avals
        ]

    def run(self, dev_in, zeros):
        return self.fn(*dev_in, *zeros)


def _get_runner(niter=1):
    if niter not in _RUNNERS:
        _RUNNERS[niter] = _Runner(niter)
    return _RUNNERS[niter]


# ------------------------------------------------------------- host prep ----


def _prep_core_inputs(emb, title, jidx, tidx, maskf, maskb, core):
    rows = slice(core * R, (core + 1) * R)
    # dma_gather consumes flat index list l where out[p, k, :] = tbl[l[k*128+p]];
    # list element i lives at idx tile [i % 16, i // 16], replicated x8 rows.
    jl = jidx[rows].reshape(G, 128, K).transpose(0, 2, 1).reshape(G, 1024)
    jw = np.ascontiguousarray(
        jl.reshape(G, 64, 16).transpose(2, 0, 1).reshape(16, G * 64)
    )
    jidx16 = np.tile(jw, (8, 1)).astype(np.int16)

    def _pmajor(a):
        return np.ascontiguousarray(
            a[rows].reshape(G, 128, K).transpose(1, 0, 2).reshape(128, G * K)
        )

    return {
        "emb": emb,
        "title": title,
        "focal": np.ascontiguousarray(emb[rows]),
        "jidx16": jidx16,
        "tidx": _pmajor(tidx),
        "maskf": _pmajor(maskf),
        "maskb": _pmajor(maskb),
    }


def _host_inputs(inputs, title_emb_mat, nbr_batch, nbr_job, nbr_title, nbr_mask):
    inputs = np.asarray(inputs, dtype=np.float32)
    title_emb_mat = np.ascontiguousarray(np.asarray(title_emb_mat, dtype=np.float32))
    emb = np.ascontiguousarray(inputs.reshape(B * P, D))
    jidx = (
        np.asarray(nbr_batch, dtype=np.int64) * P + np.asarray(nbr_job, dtype=np.int64)
    ).reshape(B * P, K)
    tidx = np.asarray(nbr_title, dtype=np.int32).reshape(B * P, K)
    m = np.asarray(nbr_mask, dtype=np.int32).reshape(B * P, K)
    maskf = m.astype(np.float32)
    maskb = ((m - 1) * np.float32(1e9)).astype(np.float32)
    return [
        _prep_core_inputs(emb, title_emb_mat, jidx, tidx, maskf, maskb, c)
        for c in range(NCORES)
    ]


# ------------------------------------------------------------ public API ----


def kernel(inputs, title_emb_mat, nbr_batch, nbr_job, nbr_title, nbr_mask):
    in_maps = _host_inputs(
        inputs, title_emb_mat, nbr_batch, nbr_job, nbr_title, nbr_mask
    )
    runner = _get_runner(1)
    dev_in = runner.place_inputs(in_maps)
    outs = runner.run(dev_in, runner.make_zeros())
    out_full = np.asarray(outs[runner.out_names.index("out")])
    return np.ascontiguousarray(out_full)


def bench(in_maps, niters=(1, 5), reps=6):
    """Measure per-pass device time via on-device iteration scaling.

    Returns (per_pass_ns, details). RPC/dispatch overhead cancels in the
    (t_hi - t_lo) / (n_hi - n_lo) slope; each sample reuses device-resident
    inputs and pre-placed donated zero buffers.
    """
    results = {}
    for ni in niters:
        runner = _get_runner(ni)
        dev_in = runner.place_inputs(in_maps)
        zeros = [runner.make_zeros() for _ in range(reps + 1)]
        # warmup (compiles on first run)
        out = runner.run(dev_in, zeros[0])
        self_block = [o.block_until_ready() for o in out]
        t0 = time.perf_counter()
        outs = []
        for r in range(reps):
            outs.append(runner.run(dev_in, zeros[r + 1]))
        for o in outs[-1]:
            o.block_until_ready()
        dt = (time.perf_counter() - t0) / reps
        results[ni] = dt
        print(f"  niter={ni}: {dt * 1e3:.3f} ms/exec", flush=True)
    ni_lo, ni_hi = min(niters), max(niters)
    per_pass = (results[ni_hi] - results[ni_lo]) / (ni_hi - ni_lo)
    return per_pass * 1e9, results


# revision 10
# speedup vs baseline: 1.6291x; 1.6291x over previous
"""HAN metapath-attention kernel for 8 Trainium2 NeuronCores.

Problem (per reference):
    inputs        [B=512, P=64, D=512] f32   job embeddings
    title_emb_mat [T=50000, D=512]     f32   title table
    nbr_batch/job/title/mask [B, P, K=8] i32 neighbor indices + mask

    ref_embs = inputs[nbr_batch, nbr_job]           # gather over full batch
    raw_s    = einsum('bpd,bpkd->bpk', inputs, ref_embs)
    sim      = softmax(where(mask, raw_s, -1e9)) * mask
    out      = concat([inputs, einsum('bpk,bpkt->bpt', sim, title[nbr_title])], -1)
    returns [B*P, 2D]

Sharding: data-parallel over flat rows r = b*P + p; core c owns rows
[c*4096, (c+1)*4096). `inputs` (flattened to a [32768, 512] gather table) and
`title_emb_mat` are replicated to every core; neighbor indices are
host-flattened to global row ids so gathers need no cross-core traffic.

Per core, per group of 128 focal rows:
  - focal tile F loaded sequentially (HWDGE)
  - all 8 neighbor job rows per focal gathered in ONE dma_gather
    (1024 int16 indices; job row ids fit int16 since B*P == 32768)
  - 8 title-row gathers via indirect_dma_start (int32 ids, one row/partition)
  - dots via fused scalar_tensor_tensor (product + row-reduce accum_out)
  - masked softmax over k in the free dim (mask folded in as +0/-1e9 bias)
  - weighted title sum via fused scalar_tensor_tensor (mult+add) chain
  - store [128, 0:512]=F and [128, 512:1024]=acc
"""

import os
import sys
import time

if "/opt/trn_rl_repo" not in sys.path:
    sys.path.insert(0, "/opt/trn_rl_repo")

import numpy as np

import concourse.bacc as bacc
import concourse.bass as bass
import concourse.tile as tile
from concourse import mybir

B, P, K, D, T = 512, 64, 8, 512, 50000
NCORES = 8
R = B * P // NCORES  # 4096 focal rows per core
G = R // 128  # 32 groups of 128 focal rows
F32 = mybir.dt.float32
I32 = mybir.dt.int32
I16 = mybir.dt.int16


def _build_program(niter=1, u_pad=B * P):
    nc = bacc.Bacc("TRN2", target_bir_lowering=False, debug=False)

    emb = nc.dram_tensor("emb", [B * P, D], F32, kind="ExternalInput")
    # per-core compacted title table (distinct titles referenced by this core,
    # always <= 32768 slots referenced, so local ids fit int16)
    title = nc.dram_tensor("title", [u_pad, D], F32, kind="ExternalInput")
    focal = nc.dram_tensor("focal", [R, D], F32, kind="ExternalInput")
    jidx16 = nc.dram_tensor("jidx16", [128, G * 64], I16, kind="ExternalInput")
    tidx16 = nc.dram_tensor("tidx16", [128, G * 64], I16, kind="ExternalInput")
    maskf = nc.dram_tensor("maskf", [128, G * K], F32, kind="ExternalInput")
    maskb = nc.dram_tensor("maskb", [128, G * K], F32, kind="ExternalInput")
    outp = nc.dram_tensor("out", [R, 2 * D], F32, kind="ExternalOutput")

    with tile.TileContext(nc) as tc:
        with (
            tc.tile_pool(name="idxp", bufs=1) as idxp,
            tc.tile_pool(name="fp", bufs=4) as fp,
            tc.tile_pool(name="jp", bufs=3) as jp,
            tc.tile_pool(name="tp", bufs=3) as tp,
            tc.tile_pool(name="wp", bufs=3) as wp,
            tc.tile_pool(name="sp", bufs=4) as sp,
        ):
            jt = idxp.tile([128, G * 64], I16)
            tt = idxp.tile([128, G * 64], I16)
            mf = idxp.tile([128, G * K], F32)
            mb = idxp.tile([128, G * K], F32)
            nc.sync.dma_start(out=jt[:], in_=jidx16[:])
            nc.sync.dma_start(out=tt[:], in_=tidx16[:])
            nc.sync.dma_start(out=mf[:], in_=maskf[:])
            nc.sync.dma_start(out=mb[:], in_=maskb[:])

            for _ in range(niter):
                for g in range(G):
                    rows = slice(g * 128, (g + 1) * 128)
                    F = fp.tile([128, D], F32, tag="F")
                    nc.sync.dma_start(out=F[:], in_=focal[rows, :])

                    J = jp.tile([128, K, D], F32, tag="J")
                    nc.gpsimd.dma_gather(
                        J[:], emb[:], jt[:, g * 64 : (g + 1) * 64], 1024, 1024, D
                    )

                    Tt = tp.tile([128, K, D], F32, tag="T")
                    nc.gpsimd.dma_gather(
                        Tt[:], title[:], tt[:, g * 64 : (g + 1) * 64], 1024, 1024, D
                    )

                    # dots[:, k] = sum_d F * J_k (fused product + row-reduce)
                    dots = sp.tile([128, K], F32, tag="dots")
                    prod = wp.tile([128, D], F32, tag="prod")
                    for k in range(K):
                        nc.vector.scalar_tensor_tensor(
                            out=prod[:],
                            in0=F[:],
                            scalar=1.0,
                            in1=J[:, k, :],
                            op0=mybir.AluOpType.mult,
                            op1=mybir.AluOpType.mult,
                            accum_out=dots[:, k : k + 1],
                        )
                    # masked logits = dots + maskbias (0 or -1e9)
                    logits = sp.tile([128, K], F32, tag="logits")
                    nc.vector.tensor_tensor(
                        out=logits[:],
                        in0=dots[:],
                        in1=mb[:, g * K : (g + 1) * K],
                        op=mybir.AluOpType.add,
                    )

                    negM = sp.tile([128, 1], F32, tag="negM")
                    nc.vector.tensor_reduce(
                        out=negM[:],
                        in_=logits[:],
                        axis=mybir.AxisListType.X,
                        op=mybir.AluOpType.max,
                        negate=True,
                    )
                    e = sp.tile([128, K], F32, tag="e")
                    nc.scalar.activation(
                        out=e[:],
                        in_=logits[:],
                        func=mybir.ActivationFunctionType.Exp,
                        bias=negM[:, 0:1],
                        scale=1.0,
                    )
                    ssum = sp.tile([128, 1], F32, tag="ssum")
                    nc.vector.tensor_reduce(
                        out=ssum[:],
                        in_=e[:],
                        axis=mybir.AxisListType.X,
                        op=mybir.AluOpType.add,
                    )
                    rr = sp.tile([128, 1], F32, tag="rr")
                    nc.vector.reciprocal(out=rr[:], in_=ssum[:])
                    sim = sp.tile([128, K], F32, tag="sim")
                    nc.vector.scalar_tensor_tensor(
                        out=sim[:],
                        in0=e[:],
                        scalar=rr[:, 0:1],
                        in1=mf[:, g * K : (g + 1) * K],
                        op0=mybir.AluOpType.mult,
                        op1=mybir.AluOpType.mult,
                    )

                    acc = wp.tile([128, D], F32, tag="acc")
                    nc.vector.tensor_scalar_mul(
                        out=acc[:], in0=Tt[:, 0, :], scalar1=sim[:, 0:1]
                    )
                    for k in range(1, K):
                        nc.vector.scalar_tensor_tensor(
                            out=acc[:],
                            in0=Tt[:, k, :],
                            scalar=sim[:, k : k + 1],
                            in1=acc[:],
                            op0=mybir.AluOpType.mult,
                            op1=mybir.AluOpType.add,
                        )

                    nc.sync.dma_start(out=outp[rows, 0:D], in_=F[:])
                    nc.sync.dma_start(out=outp[rows, D : 2 * D], in_=acc[:])

    nc.finalize()
    return nc


# ---------------------------------------------------------------- runner ----

_RUNNERS = {}


class _Runner:
    """Caches the sharded jit executable for one program variant so repeated
    executions skip retracing/recompiling (adapted from
    concourse.bass2jax.run_bass_via_pjrt's multi-core branch)."""

    def __init__(self, niter, u_pad):
        import jax
        from jax.experimental.shard_map import shard_map
        from jax.sharding import Mesh, NamedSharding, PartitionSpec

        from concourse import mybir as _mb
        from concourse.bass2jax import (
            _bass_exec_p,
            install_neuronx_cc_hook,
            partition_id_tensor,
        )

        install_neuronx_cc_hook()
        self.jax = jax
        nc = _build_program(niter, u_pad)
        self.nc = nc

        in_names, out_names, out_avals = [], [], []
        partition_name = (
            nc.partition_id_tensor.name if nc.partition_id_tensor else None
        )
        for alloc in nc.m.functions[0].allocations:
            if not isinstance(alloc, _mb.MemoryLocationSet):
                continue
            name = alloc.memorylocations[0].name
            if alloc.kind == "ExternalInput":
                if name != partition_name:
                    in_names.append(name)
            elif alloc.kind == "ExternalOutput":
                shape = tuple(alloc.tensor_shape)
                dtype = _mb.dt.np(alloc.dtype)
                out_names.append(name)
                out_avals.append(jax.core.ShapedArray(shape, dtype))

        self.in_names = list(in_names)
        self.out_names = out_names
        self.out_avals = out_avals
        n_params = len(in_names)
        n_outs = len(out_avals)

        bind_in_names = list(in_names) + list(out_names)
        if partition_name is not None:
            bind_in_names.append(partition_name)

        def _body(*args):
            operands = list(args)
            if partition_name is not None:
                operands.append(partition_id_tensor())
            outs = _bass_exec_p.bind(
                *operands,
                out_avals=tuple(out_avals),
                in_names=tuple(bind_in_names),
                out_names=tuple(out_names),
                lowering_input_output_aliases=(),
                sim_require_finite=True,
                sim_require_nnan=True,
                nc=nc,
            )
            return tuple(outs)

        devices = jax.devices()[:NCORES]
        mesh = Mesh(np.asarray(devices), ("core",))
        self.mesh = mesh
        self.sharding = NamedSharding(mesh, PartitionSpec("core"))
        in_specs = (PartitionSpec("core"),) * (n_params + n_outs)
        out_specs = (PartitionSpec("core"),) * n_outs
        donate = tuple(range(n_params, n_params + n_outs))
        self.fn = jax.jit(
            shard_map(
                _body,
                mesh=mesh,
                in_specs=in_specs,
                out_specs=out_specs,
                check_rep=False,
            ),
            donate_argnums=donate,
            keep_unused=True,
        )

    def place_inputs(self, in_maps):
        """device_put the concatenated per-core inputs once."""
        concat = [
            np.concatenate([np.asarray(m[name]) for m in in_maps], axis=0)
            for name in self.in_names
        ]
        return [self.jax.device_put(a, self.sharding) for a in concat]

    def make_zeros(self):
        return [
            self.jax.device_put(
                np.zeros((NCORES * av.shape[0], *av.shape[1:]), av.dtype),
                self.sharding,
            )
            for av in self.out_avals
        ]

    def run(self, dev_in, zeros):
        return self.fn(*dev_in, *zeros)


def _get_runner(niter, u_pad):
    key = (niter, u_pad)
    if key not in _RUNNERS:
        _RUNNERS[key] = _Runner(niter, u_pad)
    return _RUNNERS[key]


# ------------------------------------------------------------- host prep ----


def _wrap16(lists):
    """[G, 1024] flat gather lists -> [128, G*64] int16 idx tile.

    dma_gather consumes flat list l where out[p, k, :] = tbl[l[k*128+p]];
    list element i lives at idx tile [i % 16, i // 16], replicated x8 rows.
    """
    g = lists.shape[0]
    w = np.ascontiguousarray(lists.reshape(g, 64, 16).transpose(2, 0, 1).reshape(16, g * 64))
    return np.tile(w, (8, 1)).astype(np.int16)


def _prep_core_inputs(emb, title, jidx, tidx, maskf, maskb, core, u_pad):
    rows = slice(core * R, (core + 1) * R)
    jl = jidx[rows].reshape(G, 128, K).transpose(0, 2, 1).reshape(G, 1024)
    jidx16 = _wrap16(jl)

    # compact this core's referenced titles: <= 32768 distinct -> int16 local ids
    tids = tidx[rows]  # [R, K] int32
    uniq, inv = np.unique(tids, return_inverse=True)
    assert len(uniq) <= u_pad, (len(uniq), u_pad)
    tloc = np.zeros((u_pad, D), dtype=np.float32)
    tloc[: len(uniq)] = title[uniq]
    tl = inv.reshape(R, K).reshape(G, 128, K).transpose(0, 2, 1).reshape(G, 1024)
    tidx16 = _wrap16(tl)

    def _pmajor(a):
        return np.ascontiguousarray(
            a[rows].reshape(G, 128, K).transpose(1, 0, 2).reshape(128, G * K)
        )

    return {
        "emb": emb,
        "title": tloc,
        "focal": np.ascontiguousarray(emb[rows]),
        "jidx16": jidx16,
        "tidx16": tidx16,
        "maskf": _pmajor(maskf),
        "maskb": _pmajor(maskb),
    }


def _host_inputs(inputs, title_emb_mat, nbr_batch, nbr_job, nbr_title, nbr_mask):
    inputs = np.asarray(inputs, dtype=np.float32)
    title_emb_mat = np.ascontiguousarray(np.asarray(title_emb_mat, dtype=np.float32))
    emb = np.ascontiguousarray(inputs.reshape(B * P, D))
    jidx = (
        np.asarray(nbr_batch, dtype=np.int64) * P + np.asarray(nbr_job, dtype=np.int64)
    ).reshape(B * P, K)
    tidx = np.asarray(nbr_title, dtype=np.int32).reshape(B * P, K)
    m = np.asarray(nbr_mask, dtype=np.int32).reshape(B * P, K)
    maskf = m.astype(np.float32)
    maskb = ((m - 1) * np.float32(1e9)).astype(np.float32)
    # one shared padded size for all cores (SPMD: one program, one shape)
    nuniq = [
        len(np.unique(tidx[c * R : (c + 1) * R])) for c in range(NCORES)
    ]
    u_pad = -(-max(nuniq) // 512) * 512
    in_maps = [
        _prep_core_inputs(emb, title_emb_mat, jidx, tidx, maskf, maskb, c, u_pad)
        for c in range(NCORES)
    ]
    return in_maps, u_pad


# ------------------------------------------------------------ public API ----


def kernel(inputs, title_emb_mat, nbr_batch, nbr_job, nbr_title, nbr_mask):
    in_maps, u_pad = _host_inputs(
        inputs, title_emb_mat, nbr_batch, nbr_job, nbr_title, nbr_mask
    )
    runner = _get_runner(1, u_pad)
    dev_in = runner.place_inputs(in_maps)
    outs = runner.run(dev_in, runner.make_zeros())
    out_full = np.asarray(outs[runner.out_names.index("out")])
    return np.ascontiguousarray(out_full)


def bench(in_maps, u_pad, niters=(1, 5), reps=6):
    """Measure per-pass device time via on-device iteration scaling.

    Returns (per_pass_ns, details). RPC/dispatch overhead cancels in the
    (t_hi - t_lo) / (n_hi - n_lo) slope; each sample reuses device-resident
    inputs and pre-placed donated zero buffers.
    """
    results = {}
    for ni in niters:
        runner = _get_runner(ni, u_pad)
        dev_in = runner.place_inputs(in_maps)
        zeros = [runner.make_zeros() for _ in range(reps + 1)]
        # warmup (compiles on first run)
        out = runner.run(dev_in, zeros[0])
        self_block = [o.block_until_ready() for o in out]
        t0 = time.perf_counter()
        outs = []
        for r in range(reps):
            outs.append(runner.run(dev_in, zeros[r + 1]))
        for o in outs[-1]:
            o.block_until_ready()
        dt = (time.perf_counter() - t0) / reps
        results[ni] = dt
        print(f"  niter={ni}: {dt * 1e3:.3f} ms/exec", flush=True)
    ni_lo, ni_hi = min(niters), max(niters)
    per_pass = (results[ni_hi] - results[ni_lo]) / (ni_hi - ni_lo)
    return per_pass * 1e9, results
